# revision 1
# baseline (speedup 1.0000x reference)
"""Trainium (trn2) kernel for CurvedRoIExtractor (nn_CurvedRoIExtractor_28295244546862).

kernel(**inputs) takes the FULL inputs (as produced by setup_inputs()) and
returns the FULL output [2, 256, 256, 3, 16] f32.

Sharding: 8 cores = (batch b in {0,1}) x (64-roi quarter).  The core's
feature maps are pre-transposed on the host to a channel-last fp16 table
[34112, 256] (levels concatenated, level-3 zero-padded to 512 rows).
Levels 0-2: for every sample point the device fetches the two
ADJACENT-pixel pairs (x0,x1)@y0 and (x0,x1)@y1 with nc.gpsimd.dma_gather
— one 1 KB descriptor per pair (elem_size=512 fp16 elems, elem_step=256:
overlapping pair windows on the row grid).  Token order t = g64*128 +
tb*64 + j puts a 64-point group's top pairs in partitions 0-63, bottom
pairs in 64-127; the bilinear weighted sum runs on TensorE as matmuls
with a two-band masked lhsT[q, j] = (q%64==j) * w_{tb(q), side}[j]
(host-precomputed weights, lhsT built on DVE as mask x broadcast).
Level 3 (20x20) is computed DENSELY: the whole level-3 table sits in
SBUF and each 64-point group adds 4 matmuls with a host-precomputed
sparse-in-dense weight matrix W3[pixel, point] — no per-point gather
traffic for it.  All 10 matmuls per (group, chain) accumulate in PSUM;
PSUM (f32) is staged to fp16 on the Scalar engine into [128, 512] tiles
(1 KB DMA packets) and written out tile-major; the host reorders +
upcasts.

A 16-descriptor warmup gather (indices from the const-0 pool, no input
dependency) launches right after the gpsimd library load to absorb the
cold-ucode penalty while the idx tables stream in.
"""

from contextlib import ExitStack

import numpy as np

import concourse.bass as bass
import concourse.mybir as mybir
import concourse.tile as tile
from concourse import library_config
from concourse.bass_utils import run_bass_kernel_spmd
from concourse.tile import add_dep_helper

F32 = mybir.dt.float32
F16 = mybir.dt.float16
I16 = mybir.dt.int16
AOP = mybir.AluOpType

# (W, H, base row) of each feature level inside the concatenated table
LEVELS = [
    (160, 160, 0),
    (80, 80, 25600),
    (40, 40, 32000),
    (20, 20, 33600),
]
NGLVL = 3             # levels gathered per point; level 3 handled densely
ROWS = 34112          # 34000 + 112 zero pad rows (level-3 tile pad)
C = 256               # channels
BS = 2
NROI_TOTAL = 256
WP = 16
OUT_H = 3
NPTS = 3072           # per core: 64 rois * 3 * 16
NG = 4                # 128-token groups per gather (-> 1024-idx gathers)
NCHUNK = NPTS // (NG * 128)   # 6 chunks of 512 points
NSEG = NCHUNK * NGLVL  # gather segments (18)
ICOLS = NSEG * 64 + 32 + 8  # chunk idx + t3 idx (512) + w3 idx (128)
NOUT = NPTS // 256    # 12 output tiles of [128, 512]
NG64 = NPTS // 64     # 48 64-point groups
W3COLS = NG64 * 4 * 64  # 12288


def _fix_waits(nc, max_waits=1):
    """The walrus build in this env rejects >1 sem wait per instruction;
    spill extras onto preceding NOPs on the same engine."""
    for func in nc.m.functions:
        for bb in func.blocks:
            insts = bb.instructions
            for ins in list(insts):
                si = ins.sync_info
                if si is None:
                    continue
                w = list(si.on_wait)
                if len(w) > max_waits:
                    si.on_wait = w[:max_waits]
                    pos = insts.index(ins)
                    extra = w[max_waits:]
                    for k in range(0, len(extra), max_waits):
                        nop = mybir.InstNoOp(
                            name=f"{ins.name}-wf{k}",
                            engine=ins.engine,
                            bass_nofuse=True,
                            sync_info=mybir.SyncInfo(
                                on_wait=extra[k : k + max_waits], on_update=[]
                            ),
                        )
                        insts.insert(pos, nop)
                        pos += 1


def _build_kernel(fix=True):
    """Per-core program.  See module docstring for the layout."""
    nc = bass.Bass("TRN2", target_bir_lowering=False, num_devices=8,
                   num_swdge_queues=4)
    tf = nc.dram_tensor("tfeats", [ROWS, C], F16, kind="ExternalInput")
    idxd = nc.dram_tensor("idx", [128, ICOLS], I16, kind="ExternalInput")
    wmd = nc.dram_tensor("wm", [128, 64 + NCHUNK * NGLVL * 16], F16,
                         kind="ExternalInput")
    w3d = nc.dram_tensor("w3", [128, W3COLS], F16, kind="ExternalInput")
    outd = nc.dram_tensor("out", [NOUT, 128, 2 * C], F16,
                          kind="ExternalOutput")
    tf_h = tf[:].tensor

    with tile.TileContext(nc) as tc, ExitStack() as ctx:
        prep = ctx.enter_context(tc.tile_pool(name="prep", bufs=1))
        gpool = ctx.enter_context(tc.tile_pool(name="g", bufs=4))
        opool = ctx.enter_context(tc.tile_pool(name="o", bufs=1))
        ppool = ctx.enter_context(tc.tile_pool(name="ps", bufs=6, space="PSUM"))

        nc.gpsimd.load_library(library_config.attnmlp)

        idxt = prep.tile([128, ICOLS], I16, tag="idx")
        wmt = prep.tile([128, 64 + NCHUNK * NGLVL * 16], F16, tag="wm")
        w3t = prep.tile([128, 1, W3COLS], F16, tag="w3")
        t3 = prep.tile([128, 4, C], F16, tag="t3")
        warm = prep.tile([128, 1, C], F16, tag="warm")

        reg16 = nc.gpsimd.to_reg(16)
        reg128 = nc.gpsimd.to_reg(128)
        reg512 = nc.gpsimd.to_reg(512)
        reg1024 = nc.gpsimd.to_reg(NG * 256)

        # warmup gather: no input dependency (idx = const-0 pool), absorbs
        # the cold-ucode cost while the idx tables stream in
        zidx = nc.const_aps.tensor(0.0, [128, 1], F32).bitcast(I16)[:, 0:1]
        nc.gpsimd.dma_gather(
            out_ap=warm[:],
            in_ap=bass.AP(tf_h, 0, [[C, 16], [1, C]]),
            idxs_ap=zidx,
            num_idxs=16,
            num_idxs_reg=reg16,
            elem_size=C,
            queue_num=0,
        )

        # chunk0's idx slice first so its gathers can launch ASAP
        nc.sync.dma_start(idxt[:, 0:192], idxd[:, 0:192])
        nc.sync.dma_start(idxt[:, 192:ICOLS], idxd[:, 192:ICOLS])
        nc.sync.dma_start(wmt[:], wmd[:])
        mask = wmt[:, 0:64]

        # lhsT tiles for every chunk, built upfront on DVE:
        # lt[q, l*16 + g64*2 + s, j] = mask[q, j] * w[q, chunk-col]
        lts = []
        for ch in range(NCHUNK):
            lt = prep.tile([128, NGLVL * 16, 64], F16, tag=f"lt{ch}")
            nc.vector.tensor_tensor(
                lt[:],
                mask.unsqueeze(1).to_broadcast([128, NGLVL * 16, 64]),
                wmt[:, 64 + ch * NGLVL * 16 : 64 + (ch + 1) * NGLVL * 16]
                    .unsqueeze(2).to_broadcast([128, NGLVL * 16, 64]),
                AOP.mult,
            )
            lts.append(lt)

        prev_mm = None
        first_loads_issued = False
        for ch in range(NCHUNK):
            gts = []
            for l in range(NGLVL):
                W, H, base = LEVELS[l]
                gt = gpool.tile([128, NG * 2, 2 * C], F16, tag=f"g{l}")
                seg = ch * NGLVL + l
                # overlapping pair window: row stride C, window 2*C
                in_ap = bass.AP(tf_h, base * C, [[C, W * H - 1], [1, 2 * C]])
                nc.gpsimd.dma_gather(
                    out_ap=gt[:],
                    in_ap=in_ap,
                    idxs_ap=idxt[:, seg * 64 : (seg + 1) * 64],
                    num_idxs=NG * 256,
                    num_idxs_reg=reg1024,
                    elem_size=2 * C,
                    elem_step=C,
                    queue_num=seg % 4,
                )
                gts.append(gt)
            if not first_loads_issued:
                # level-3 table + dense weights, loaded once via gathers
                first_loads_issued = True
                b3 = LEVELS[3][2]
                nc.gpsimd.dma_gather(
                    out_ap=t3[:],
                    in_ap=bass.AP(tf_h, b3 * C, [[C, 512], [1, C]]),
                    idxs_ap=idxt[:, NSEG * 64 : NSEG * 64 + 32],
                    num_idxs=512,
                    num_idxs_reg=reg512,
                    elem_size=C,
                    queue_num=2,
                )
                w3_h = w3d[:].tensor
                for wh in range(2):
                    half = W3COLS // 2
                    nc.gpsimd.dma_gather(
                        out_ap=w3t[:, :, wh * half : (wh + 1) * half],
                        in_ap=bass.AP(w3_h, wh * half,
                                      [[W3COLS, 128], [1, half]]),
                        idxs_ap=idxt[:, NSEG * 64 + 32 : NSEG * 64 + 40],
                        num_idxs=128,
                        num_idxs_reg=reg128,
                        elem_size=half,
                        elem_step=W3COLS,
                        queue_num=2 + wh,
                    )
            lt = lts[ch]
            for tpair in range(2):     # output tile = 2 point-pairs = 256 pts
                so = opool.tile([128, 2 * C], F16, tag=f"so{ch * 2 + tpair}")
                for h in range(2):
                    pair = tpair * 2 + h
                    ps = ppool.tile([128, C], F32, tag="ps")
                    for half in range(2):
                        g64 = pair * 2 + half
                        g64g = ch * 8 + g64
                        k = 0
                        nmm = 2 * NGLVL + 4
                        for l in range(NGLVL):
                            for s in range(2):
                                mm = nc.tensor.matmul(
                                    ps[64 * half : 64 * half + 64, :],
                                    lt[:, l * 16 + g64 * 2 + s, :],
                                    gts[l][:, g64, s * C : (s + 1) * C],
                                    start=(k == 0),
                                    stop=(k == nmm - 1),
                                )
                                # accumulation chains sharing a PSUM bank
                                # must not interleave -> force PE order
                                if prev_mm is not None:
                                    add_dep_helper(mm.ins, prev_mm.ins,
                                                   sync=False)
                                prev_mm = mm
                                k += 1
                        for kt in range(4):   # dense level-3
                            off = (g64g * 4 + kt) * 64
                            mm = nc.tensor.matmul(
                                ps[64 * half : 64 * half + 64, :],
                                w3t[:, 0, off : off + 64],
                                t3[:, kt, :],
                                start=(k == 0),
                                stop=(k == nmm - 1),
                            )
                            add_dep_helper(mm.ins, prev_mm.ins, sync=False)
                            prev_mm = mm
                            k += 1
                    nc.scalar.activation(so[:, h * C : (h + 1) * C], ps[:],
                                         mybir.ActivationFunctionType.Copy)
                nc.sync.dma_start(outd[ch * 2 + tpair], so[:])

    mybir.codegen_inst_isa_subclasses(nc)
    if fix:
        _fix_waits(nc)
    return nc


# ---------------------------------------------------------------------------
# Host-side prep

def _wrap128(flat):
    """Token-order idx list -> wrapped [16, n/16] replicated to [128, ...]."""
    w = flat.reshape(-1, 16).T.astype(np.int16)
    return np.tile(w, (8, 1))


def _host_prep_points(center_b, boundary_b, roi0, nroi):
    """Returns (idx [128, ICOLS] i16, wm [128, .] f16, w3 [128, .] f16)."""
    bp = boundary_b[roi0 : roi0 + nroi]      # [nroi, Wp, 4]
    cp = center_b[roi0 : roi0 + nroi]        # [nroi, Wp, 2]
    sp = np.stack([bp[..., 0:2], cp, bp[..., 2:4]], axis=1)  # [nroi,3,Wp,2]
    gx = np.ascontiguousarray(sp[..., 0].transpose(1, 2, 0)).reshape(-1)
    gy = np.ascontiguousarray(sp[..., 1].transpose(1, 2, 0)).reshape(-1)
    gx = gx.astype(np.float32)
    gy = gy.astype(np.float32)

    q = np.arange(128)
    jj = q % 64
    tb = q // 64           # 0 = top pair (y0), 1 = bottom pair (y1)

    idx = np.zeros((128, ICOLS), np.int16)
    wm = np.zeros((128, 64 + NCHUNK * NGLVL * 16), np.float16)
    wm[:, 0:64] = (q[:, None] % 64 == np.arange(64)[None, :])

    def lvl_geom(W, H):
        x = ((gx + np.float32(1.0)) * np.float32(0.5)) * np.float32(W - 1)
        y = ((gy + np.float32(1.0)) * np.float32(0.5)) * np.float32(H - 1)
        x0 = np.floor(x)
        y0 = np.floor(y)
        return x0.astype(np.int32), y0.astype(np.int32), x - x0, y - y0

    for l in range(NGLVL):
        W, H, base = LEVELS[l]
        x0, y0, wx, wy = lvl_geom(W, H)
        it = y0 * W + x0
        ib = it + W
        w00 = (1 - wx) * (1 - wy)
        w10 = wx * (1 - wy)
        w01 = (1 - wx) * wy
        w11 = wx * wy
        for ch in range(NCHUNK):
            seg = ch * NGLVL + l
            tok = np.empty((NG * 2, 128), np.int32)
            for g64 in range(8):
                p64 = ch * 512 + g64 * 64 + np.arange(64)
                tok[g64, :64] = it[p64]
                tok[g64, 64:] = ib[p64]
            idx[:, seg * 64 : (seg + 1) * 64] = _wrap128(tok.reshape(-1))
            for g64 in range(8):
                p128 = ch * 512 + g64 * 64 + jj
                ws0 = np.where(tb == 0, w00[p128], w01[p128])
                ws1 = np.where(tb == 0, w10[p128], w11[p128])
                col = 64 + ch * NGLVL * 16 + l * 16 + g64 * 2
                wm[:, col] = ws0.astype(np.float16)
                wm[:, col + 1] = ws1.astype(np.float16)

    # t3 / w3 bootstrap idx
    idx[:, NSEG * 64 : NSEG * 64 + 32] = _wrap128(np.arange(512))
    idx[:, NSEG * 64 + 32 : NSEG * 64 + 40] = _wrap128(np.arange(128))

    # dense level-3 weights: W3[pix, pt] (512 pix rows with pad, 3072 pts)
    W, H = 20, 20
    x0, y0, wx, wy = lvl_geom(W, H)
    w3full = np.zeros((512, NPTS), np.float32)
    pts = np.arange(NPTS)
    for dy, dx, wgt in ((0, 0, (1 - wx) * (1 - wy)), (0, 1, wx * (1 - wy)),
                        (1, 0, (1 - wx) * wy), (1, 1, wx * wy)):
        w3full[(y0 + dy) * W + (x0 + dx), pts] = wgt
    # w3[p, (g64*4 + k)*64 + j] = w3full[k*128 + p, g64*64 + j]
    w3 = np.ascontiguousarray(
        w3full.reshape(4, 128, NG64, 64)      # [k, p, g64, j]
        .transpose(1, 2, 0, 3)                # [p, g64, k, j]
        .reshape(128, W3COLS)).astype(np.float16)
    return idx, wm, w3


def _host_tfeats(feats_b_list):
    parts = [np.ascontiguousarray(f.reshape(f.shape[0], -1).T)
             for f in feats_b_list]
    tfx = np.concatenate(parts, axis=0)
    pad = ROWS - tfx.shape[0]
    tfx = np.concatenate([tfx, np.zeros((pad, C), tfx.dtype)], axis=0)
    return np.ascontiguousarray(tfx.astype(np.float16))


_CACHE = {}


def _get_nc():
    if "nc" not in _CACHE:
        _CACHE["nc"] = _build_kernel()
    return _CACHE["nc"]


def kernel(feats0, feats1, feats2, feats3, center_points, boundary_points,
           _want_trace=False, _trace_dir=None):
    feats0 = np.asarray(feats0, dtype=np.float32)
    feats1 = np.asarray(feats1, dtype=np.float32)
    feats2 = np.asarray(feats2, dtype=np.float32)
    feats3 = np.asarray(feats3, dtype=np.float32)
    center_points = np.asarray(center_points, dtype=np.float32)
    boundary_points = np.asarray(boundary_points, dtype=np.float32)

    nc = _get_nc()
    tfeats = [
        _host_tfeats([feats0[b], feats1[b], feats2[b], feats3[b]])
        for b in range(BS)
    ]
    nroi = NROI_TOTAL // 4  # 64 rois per core
    in_maps = []
    for core in range(8):
        b = core // 4
        roi0 = (core % 4) * nroi
        idx, wm, w3 = _host_prep_points(
            center_points[b], boundary_points[b], roi0, nroi)
        in_maps.append(
            {"tfeats": tfeats[b], "idx": idx, "wm": wm, "w3": w3})

    kwargs = {}
    if _want_trace:
        kwargs = {"trace": True}
        if _trace_dir is not None:
            kwargs["tmpdir"] = _trace_dir
    res = run_bass_kernel_spmd(nc, in_maps, core_ids=list(range(8)), **kwargs)

    out = np.empty((BS, NROI_TOTAL, C, OUT_H, WP), np.float32)
    for core in range(8):
        b = core // 4
        roi0 = (core % 4) * nroi
        dev = res.results[core]["out"]          # [12, 128, 512] f16
        pts = (dev.astype(np.float32)
               .reshape(NOUT, 128, 2, C)
               .transpose(0, 2, 1, 3)
               .reshape(NPTS, C))               # rows (h, w, roi')
        o = pts.reshape(OUT_H, WP, nroi, C)
        out[b, roi0 : roi0 + nroi] = o.transpose(2, 3, 0, 1)
    if _want_trace:
        return out, res
    return out



# revision 3
# speedup vs baseline: 1.1626x; 1.1626x over previous
"""Trainium (trn2) kernel for CurvedRoIExtractor (nn_CurvedRoIExtractor_28295244546862).

kernel(**inputs) takes the FULL inputs (as produced by setup_inputs()) and
returns the FULL output [2, 256, 256, 3, 16] f32.

Sharding: 8 cores = (batch b in {0,1}) x (64-roi quarter) -> 3072 sample
points per core.  Host-side, each core's points are SORTED by their
level-2 cell index; the device then processes 24 blocks of 128 sorted
points, accumulating all 4 feature levels into one PSUM tile
[128 points, 256 ch] per block:

* Levels 0/1 (160^2 / 80^2): per-point gathers from a host-built
  "row-pair" table T01[y*W+x] = [feat[y,x,:]; feat[y+1,x,:]] (fp16,
  2C per row).  ONE 2 KB descriptor (elem 4C, step 2C) fetches all 4
  bilinear corners of a point into its partition.  Per level per block:
  4 matmuls with a diagonal lhsT diag(w_k) (built on DVE as
  identity x weight-broadcast), rhs = the gathered corner-k columns.
* Levels 2/3 (40^2 / 20^2): NO gathers.  The plain channel-last tables
  (1600 + 400 rows) sit in SBUF; because points are sorted by level-2
  cell, each 128-point block touches only 1-2 aligned 128-row windows
  per level.  Per window: one matmul with a host-built sparse-in-dense
  lhsT W[row-in-window, point] (weights at the corner rows).  The
  window->chunk map is computed from the actual inputs (union over all
  8 cores, so the single SPMD program fits every core) and baked into
  the program at first kernel() call.

PSUM (f32) -> fp16 on the Scalar engine -> per-block 64 KB output DMA;
the host inverts the sort and reshapes.  Sequential table loads (t23,
w23 lhsT, idx, weights) launch at t=0 on the sync engine while the
gpsimd library loads, so the DMA engines are busy from the start.
"""

from contextlib import ExitStack

import numpy as np

import concourse.bass as bass
import concourse.mybir as mybir
import concourse.tile as tile
from concourse import library_config
from concourse.bass_utils import run_bass_kernel_spmd
from concourse.tile import add_dep_helper

F32 = mybir.dt.float32
F16 = mybir.dt.float16
I16 = mybir.dt.int16
AOP = mybir.AluOpType

C = 256
BS = 2
NROI_TOTAL = 256
WP = 16
OUT_H = 3
NPTS = 3072                # per core: 64 rois * 3 * 16
NBLK = NPTS // 128         # 24 blocks of 128 points
NSB = NBLK // 4            # 6 gather superblocks of 512 points

# levels 0/1: gathered from the row-pair table
L01 = [(160, 160, 0), (80, 80, 25600)]   # (W, H, row base in T01)
T01_ROWS = 25600 + 6400 + 1             # + 1 zero pad row
# levels 2/3: SBUF-resident plain tables
L23 = [(40, 40, 0), (20, 20, 1664)]      # (W, H, row base in T23)
T23_CHUNKS = 17                          # 1664 rows L2-pad + 400 L3 + pad
T23_ROWS = T23_CHUNKS * 128

NGCOL = 32                 # idx cols per 512-point gather (512/16)
IDXCOLS = NSB * 2 * NGCOL


def _fix_waits(nc, max_waits=1):
    """The walrus build in this env rejects >1 sem wait per instruction;
    spill extras onto preceding NOPs on the same engine."""
    for func in nc.m.functions:
        for bb in func.blocks:
            insts = bb.instructions
            for ins in list(insts):
                si = ins.sync_info
                if si is None:
                    continue
                w = list(si.on_wait)
                if len(w) > max_waits:
                    si.on_wait = w[:max_waits]
                    pos = insts.index(ins)
                    extra = w[max_waits:]
                    for k in range(0, len(extra), max_waits):
                        nop = mybir.InstNoOp(
                            name=f"{ins.name}-wf{k}",
                            engine=ins.engine,
                            bass_nofuse=True,
                            sync_info=mybir.SyncInfo(
                                on_wait=extra[k : k + max_waits], on_update=[]
                            ),
                        )
                        insts.insert(pos, nop)
                        pos += 1


def _build_kernel(winplan, fix=True):
    """Per-core program.  winplan: per block, ([L2 chunks], [L3 chunks])."""
    nwin = sum(len(w2) + len(w3) for w2, w3 in winplan)
    nc = bass.Bass("TRN2", target_bir_lowering=False, num_devices=8,
                   num_swdge_queues=4)
    t01 = nc.dram_tensor("t01", [T01_ROWS, 2 * C], F16, kind="ExternalInput")
    t23d = nc.dram_tensor("t23", [T23_CHUNKS, 128, C], F16,
                          kind="ExternalInput")
    w23d = nc.dram_tensor("w23", [128, nwin * 128], F16, kind="ExternalInput")
    idxd = nc.dram_tensor("idx", [128, IDXCOLS], I16, kind="ExternalInput")
    wtsd = nc.dram_tensor("wts", [128, NBLK * 8], F16, kind="ExternalInput")
    identd = nc.dram_tensor("ident", [128, 128], F16, kind="ExternalInput")
    outd = nc.dram_tensor("out", [NBLK, 128, C], F16, kind="ExternalOutput")
    t01_h = t01[:].tensor

    with tile.TileContext(nc) as tc, ExitStack() as ctx:
        prep = ctx.enter_context(tc.tile_pool(name="prep", bufs=1))
        g0pool = ctx.enter_context(tc.tile_pool(name="g0", bufs=3))
        g1pool = ctx.enter_context(tc.tile_pool(name="g1", bufs=3))
        ltpool = ctx.enter_context(tc.tile_pool(name="lt", bufs=2))
        opool = ctx.enter_context(tc.tile_pool(name="o", bufs=3))
        ppool = ctx.enter_context(tc.tile_pool(name="ps", bufs=6, space="PSUM"))

        nc.gpsimd.load_library(library_config.attnmlp)

        idxt = prep.tile([128, IDXCOLS], I16, tag="idx")
        wtst = prep.tile([128, NBLK * 8], F16, tag="wts")
        identt = prep.tile([128, 128], F16, tag="ident")
        t23t = prep.tile([128, T23_CHUNKS, C], F16, tag="t23")
        w23t = prep.tile([128, nwin * 128], F16, tag="w23")
        warm = prep.tile([128, 1, C], F16, tag="warm")

        reg16 = nc.gpsimd.to_reg(16)
        reg512 = nc.gpsimd.to_reg(512)

        # warmup gather: no input dependency (idx = const-0 pool), absorbs
        # the cold-ucode cost while the tables stream in
        zidx = nc.const_aps.tensor(0.0, [128, 1], F32).bitcast(I16)[:, 0:1]
        nc.gpsimd.dma_gather(
            out_ap=warm[:],
            in_ap=bass.AP(t01_h, 0, [[C, 16], [1, C]]),
            idxs_ap=zidx,
            num_idxs=16,
            num_idxs_reg=reg16,
            elem_size=C,
            queue_num=0,
        )

        # sequential loads first: they need no gpsimd and fill the ramp
        nc.sync.dma_start(idxt[:], idxd[:])
        nc.sync.dma_start(identt[:], identd[:])
        nc.sync.dma_start(wtst[:], wtsd[:])
        for k in range(T23_CHUNKS):
            nc.sync.dma_start(t23t[:, k, :], t23d[k])
        # w23 in first-use (block-major) order, split for parallelism
        wsplit = max(1, nwin // 4)
        for s in range(0, nwin, wsplit):
            e = min(nwin, s + wsplit)
            nc.sync.dma_start(w23t[:, s * 128 : e * 128],
                              w23d[:, s * 128 : e * 128])

        prev_mm = None
        wslot = 0
        for sb in range(NSB):
            gts = []
            for l in range(2):
                W, H, base = L01[l]
                pool = g0pool if l == 0 else g1pool
                gt = pool.tile([128, 4, 4 * C], F16, tag=f"g{l}")
                # one descriptor = 4C elems spanning table rows r, r+1
                in_ap = bass.AP(t01_h, base * 2 * C,
                                [[2 * C, W * H], [1, 4 * C]])
                nc.gpsimd.dma_gather(
                    out_ap=gt[:],
                    in_ap=in_ap,
                    idxs_ap=idxt[:, (sb * 2 + l) * NGCOL
                                 : (sb * 2 + l + 1) * NGCOL],
                    num_idxs=512,
                    num_idxs_reg=reg512,
                    elem_size=4 * C,
                    elem_step=2 * C,
                    queue_num=(sb * 2 + l) % 4,
                )
                gts.append(gt)

            # diagonal lhsT for the 4 blocks of this superblock:
            # lt[q, i, j] = ident[q, j] * wts[q, sb*32 + i]
            lt = ltpool.tile([128, 32, 128], F16, tag="lt")
            nc.vector.tensor_tensor(
                lt[:],
                identt[:].unsqueeze(1).to_broadcast([128, 32, 128]),
                wtst[:, sb * 32 : (sb + 1) * 32]
                    .unsqueeze(2).to_broadcast([128, 32, 128]),
                AOP.mult,
            )

            for b4 in range(4):
                b = sb * 4 + b4
                w2, w3 = winplan[b]
                nmm = 8 + len(w2) + len(w3)
                ps = ppool.tile([128, C], F32, tag="ps")
                k = 0
                for l in range(2):
                    for k4 in range(4):
                        mm = nc.tensor.matmul(
                            ps[:],
                            lt[:, b4 * 8 + l * 4 + k4, :],
                            gts[l][:, b4, k4 * C : (k4 + 1) * C],
                            start=(k == 0),
                            stop=(k == nmm - 1),
                        )
                        if prev_mm is not None:
                            add_dep_helper(mm.ins, prev_mm.ins, sync=False)
                        prev_mm = mm
                        k += 1
                for chunks in (w2, w3):
                    for c in chunks:
                        mm = nc.tensor.matmul(
                            ps[:],
                            w23t[:, wslot * 128 : (wslot + 1) * 128],
                            t23t[:, c, :],
                            start=(k == 0),
                            stop=(k == nmm - 1),
                        )
                        add_dep_helper(mm.ins, prev_mm.ins, sync=False)
                        prev_mm = mm
                        k += 1
                        wslot += 1
                ot = opool.tile([128, C], F16, tag="ot")
                nc.scalar.activation(ot[:], ps[:],
                                     mybir.ActivationFunctionType.Copy)
                nc.sync.dma_start(outd[b], ot[:])

    mybir.codegen_inst_isa_subclasses(nc)
    if fix:
        _fix_waits(nc)
    return nc


# ---------------------------------------------------------------------------
# Host-side prep

def _wrap128(flat):
    """Token-order idx list -> wrapped [16, n/16] replicated to [128, ...]."""
    w = flat.reshape(-1, 16).T.astype(np.int16)
    return np.tile(w, (8, 1))


def _points(center_b, boundary_b, roi0, nroi):
    """Flat sample-point coords (order h, w, roi) for one core."""
    bp = boundary_b[roi0 : roi0 + nroi]
    cp = center_b[roi0 : roi0 + nroi]
    sp = np.stack([bp[..., 0:2], cp, bp[..., 2:4]], axis=1)  # [nroi,3,Wp,2]
    gx = np.ascontiguousarray(sp[..., 0].transpose(1, 2, 0)).reshape(-1)
    gy = np.ascontiguousarray(sp[..., 1].transpose(1, 2, 0)).reshape(-1)
    return gx.astype(np.float32), gy.astype(np.float32)


def _lvl_geom(gx, gy, W, H):
    x = ((gx + np.float32(1.0)) * np.float32(0.5)) * np.float32(W - 1)
    y = ((gy + np.float32(1.0)) * np.float32(0.5)) * np.float32(H - 1)
    x0 = np.floor(x)
    y0 = np.floor(y)
    return x0.astype(np.int32), y0.astype(np.int32), x - x0, y - y0


class _CorePrep:
    """Per-core host data: sorted geometry for all 4 levels."""

    def __init__(self, center_b, boundary_b, roi0, nroi):
        gx, gy = _points(center_b, boundary_b, roi0, nroi)
        # sort by level-2 cell
        x2, y2, _, _ = _lvl_geom(gx, gy, 40, 40)
        self.sigma = np.argsort(y2 * 40 + x2, kind="stable")
        gx = gx[self.sigma]
        gy = gy[self.sigma]
        self.geo = {}
        for (W, H, _base) in L01 + L23:
            self.geo[W] = _lvl_geom(gx, gy, W, H)

    def corner_rows_weights(self, W):
        """[4, NPTS] corner table rows + weights (corner order 00,01,10,11)."""
        x0, y0, wx, wy = self.geo[W]
        r = y0 * W + x0
        rows = np.stack([r, r + W, r + 1, r + W + 1])
        wts = np.stack([(1 - wx) * (1 - wy), (1 - wx) * wy,
                        wx * (1 - wy), wx * wy])
        return rows, wts.astype(np.float32)

    def idx01_wts(self):
        idx = np.zeros((128, IDXCOLS), np.int16)
        wts = np.zeros((128, NBLK * 8), np.float16)
        for l, (W, H, _base) in enumerate(L01):
            x0, y0, wx, wy = self.geo[W]
            r = (y0 * W + x0).astype(np.int32)
            for sb in range(NSB):
                cs = (sb * 2 + l) * NGCOL
                idx[:, cs : cs + NGCOL] = _wrap128(r[sb * 512 : (sb + 1) * 512])
            _rows, w4 = self.corner_rows_weights(W)
            for b in range(NBLK):
                pts = slice(b * 128, (b + 1) * 128)
                for k in range(4):
                    wts[:, b * 8 + l * 4 + k] = w4[k, pts].astype(np.float16)
        return idx, wts

    def block_chunks(self, lvl):
        """Needed t23 chunks per block for level lvl (2 or 3)."""
        W, H, base = L23[lvl - 2]
        rows, wts = self.corner_rows_weights(W)
        out = []
        for b in range(NBLK):
            pts = slice(b * 128, (b + 1) * 128)
            rr = rows[:, pts] + base
            use = wts[:, pts] != 0.0
            out.append(set(np.unique(rr[use] // 128).tolist()))
        return out

    def w23(self, winplan):
        nwin = sum(len(w2) + len(w3) for w2, w3 in winplan)
        w23 = np.zeros((128, nwin * 128), np.float32)
        rw = {2: self.corner_rows_weights(40), 3: self.corner_rows_weights(20)}
        wslot = 0
        for b in range(NBLK):
            pts = slice(b * 128, (b + 1) * 128)
            for lvl, chunks in ((2, winplan[b][0]), (3, winplan[b][1])):
                rows, wts = rw[lvl]
                base = L23[lvl - 2][2]
                rr = rows[:, pts] + base           # [4, 128]
                ww = wts[:, pts]
                for c in chunks:
                    blkw = np.zeros((128, 128), np.float32)
                    rel = rr - c * 128
                    m = (rel >= 0) & (rel < 128) & (ww != 0.0)
                    jj = np.broadcast_to(np.arange(128)[None, :], (4, 128))
                    np.add.at(blkw, (rel[m], jj[m]), ww[m])
                    w23[:, wslot * 128 : (wslot + 1) * 128] = blkw
                    wslot += 1
        return w23.astype(np.float16)


def _host_t01(feats_b_list):
    """Row-pair table for levels 0/1: T01[y*W+x] = [row(y,x); row(y,x)+W]."""
    parts = []
    for f in feats_b_list:
        Cc, H, W = f.shape
        a = np.ascontiguousarray(f.reshape(Cc, -1).T)       # [H*W, C]
        bdown = np.concatenate([a[W:], np.zeros((W, Cc), a.dtype)], axis=0)
        parts.append(np.concatenate([a, bdown], axis=1))     # [H*W, 2C]
    t = np.concatenate(parts, axis=0)
    pad = T01_ROWS - t.shape[0]
    t = np.concatenate([t, np.zeros((pad, t.shape[1]), t.dtype)], axis=0)
    return np.ascontiguousarray(t.astype(np.float16))


def _host_t23(feats_b_list):
    """Plain channel-last tables for levels 2/3, chunk-padded."""
    t = np.zeros((T23_ROWS, C), np.float32)
    for f, (_W, _H, base) in zip(feats_b_list, L23):
        Cc, H, W = f.shape
        t[base : base + H * W] = f.reshape(Cc, -1).T
    return np.ascontiguousarray(
        t.reshape(T23_CHUNKS, 128, C).astype(np.float16))


_CACHE = {}


def kernel(feats0, feats1, feats2, feats3, center_points, boundary_points,
           _want_trace=False, _trace_dir=None):
    feats0 = np.asarray(feats0, dtype=np.float32)
    feats1 = np.asarray(feats1, dtype=np.float32)
    feats2 = np.asarray(feats2, dtype=np.float32)
    feats3 = np.asarray(feats3, dtype=np.float32)
    center_points = np.asarray(center_points, dtype=np.float32)
    boundary_points = np.asarray(boundary_points, dtype=np.float32)

    nroi = NROI_TOTAL // 4
    preps = []
    for core in range(8):
        b = core // 4
        roi0 = (core % 4) * nroi
        preps.append(_CorePrep(center_points[b], boundary_points[b],
                               roi0, nroi))

    # window plan: union over cores so one program fits all
    winplan = []
    per_core_chunks = [(p.block_chunks(2), p.block_chunks(3)) for p in preps]
    for b in range(NBLK):
        c2 = sorted(set().union(*[pc[0][b] for pc in per_core_chunks]))
        c3 = sorted(set().union(*[pc[1][b] for pc in per_core_chunks]))
        winplan.append((c2, c3))
    plan_key = tuple((tuple(w2), tuple(w3)) for w2, w3 in winplan)

    if _CACHE.get("key") != plan_key:
        _CACHE["nc"] = _build_kernel(winplan)
        _CACHE["key"] = plan_key
    nc = _CACHE["nc"]

    t01 = [_host_t01([feats0[b], feats1[b]]) for b in range(BS)]
    t23 = [_host_t23([feats2[b], feats3[b]]) for b in range(BS)]
    ident = np.eye(128, dtype=np.float16)

    in_maps = []
    for core in range(8):
        b = core // 4
        p = preps[core]
        idx, wts = p.idx01_wts()
        in_maps.append({
            "t01": t01[b],
            "t23": t23[b],
            "w23": p.w23(winplan),
            "idx": idx,
            "wts": wts,
            "ident": ident,
        })

    kwargs = {}
    if _want_trace:
        kwargs = {"trace": True}
        if _trace_dir is not None:
            kwargs["tmpdir"] = _trace_dir
    res = run_bass_kernel_spmd(nc, in_maps, core_ids=list(range(8)), **kwargs)

    out = np.empty((BS, NROI_TOTAL, C, OUT_H, WP), np.float32)
    for core in range(8):
        b = core // 4
        roi0 = (core % 4) * nroi
        dev = res.results[core]["out"]          # [24, 128, 256] f16
        pts_sorted = dev.astype(np.float32).reshape(NPTS, C)
        pts = np.empty_like(pts_sorted)
        pts[preps[core].sigma] = pts_sorted
        o = pts.reshape(OUT_H, WP, nroi, C)
        out[b, roi0 : roi0 + nroi] = o.transpose(2, 3, 0, 1)
    if _want_trace:
        return out, res
    return out


# revision 8
# speedup vs baseline: 1.1957x; 1.0285x over previous
"""Trainium (trn2) kernel for CurvedRoIExtractor (nn_CurvedRoIExtractor_28295244546862).

kernel(**inputs) takes the FULL inputs (as produced by setup_inputs()) and
returns the FULL output [2, 256, 256, 3, 16] f32.

Sharding: 8 cores = (batch b in {0,1}) x (64-roi quarter) -> 3072 sample
points per core.  Host-side, each core's points are SORTED by their
level-2 cell index; the device then processes 24 blocks of 128 sorted
points, accumulating all 4 feature levels into one PSUM tile
[128 points, 256 ch] per block:

* Levels 0/1 (160^2 / 80^2): per-point gathers from a host-built
  "row-pair" table T01[y*W+x] = [feat[y,x,:]; feat[y+1,x,:]] (fp16,
  2C per row).  ONE 2 KB descriptor (elem 4C, step 2C) fetches all 4
  bilinear corners of a point into its partition.  Per level per block:
  4 matmuls with a diagonal lhsT diag(w_k) (built on DVE as
  identity x weight-broadcast), rhs = the gathered corner-k columns.
* Levels 2/3 (40^2 / 20^2): NO gathers.  The plain channel-last tables
  (1600 + 400 rows) sit in SBUF; because points are sorted by level-2
  cell, each 128-point block touches only 1-2 aligned 128-row windows
  per level.  Per window: one matmul with a host-built sparse-in-dense
  lhsT W[row-in-window, point] (weights at the corner rows).  The
  window->chunk map is computed from the actual inputs (union over all
  8 cores, so the single SPMD program fits every core) and baked into
  the program at first kernel() call.

PSUM (f32) -> fp16 on the Scalar engine -> per-block 64 KB output DMA;
the host inverts the sort and reshapes.  Sequential table loads (t23,
w23 lhsT, idx, weights) launch at t=0 on the sync engine while the
gpsimd library loads, so the DMA engines are busy from the start.
"""

from contextlib import ExitStack

import numpy as np

import concourse.bass as bass
import concourse.mybir as mybir
import concourse.tile as tile
from concourse import library_config
from concourse.bass_utils import run_bass_kernel_spmd
from concourse.tile import add_dep_helper

F32 = mybir.dt.float32
F16 = mybir.dt.float16
I16 = mybir.dt.int16
AOP = mybir.AluOpType

C = 256
BS = 2
NROI_TOTAL = 256
WP = 16
OUT_H = 3
NPTS = 3072                # per core: 64 rois * 3 * 16
NBLK = NPTS // 128         # 24 blocks of 128 points
NSB = NBLK // 4            # 6 gather superblocks of 512 points

# levels 0/1: gathered from the row-pair table
L01 = [(160, 160, 0), (80, 80, 25600)]   # (W, H, row base in T01)
T01_ROWS = 25600 + 6400 + 1             # + 1 zero pad row
# levels 2/3: SBUF-resident plain tables
L23 = [(40, 40, 0), (20, 20, 1664)]      # (W, H, row base in T23)
T23_CHUNKS = 17                          # 1664 rows L2-pad + 400 L3 + pad
T23_ROWS = T23_CHUNKS * 128

NGCOL = 32                 # idx cols per 512-point gather (512/16)
IDXCOLS = NSB * 2 * NGCOL


def _fix_waits(nc, max_waits=1):
    """The walrus build in this env rejects >1 sem wait per instruction;
    spill extras onto preceding NOPs on the same engine."""
    for func in nc.m.functions:
        for bb in func.blocks:
            insts = bb.instructions
            for ins in list(insts):
                si = ins.sync_info
                if si is None:
                    continue
                w = list(si.on_wait)
                if len(w) > max_waits:
                    si.on_wait = w[:max_waits]
                    pos = insts.index(ins)
                    extra = w[max_waits:]
                    for k in range(0, len(extra), max_waits):
                        nop = mybir.InstNoOp(
                            name=f"{ins.name}-wf{k}",
                            engine=ins.engine,
                            bass_nofuse=True,
                            sync_info=mybir.SyncInfo(
                                on_wait=extra[k : k + max_waits], on_update=[]
                            ),
                        )
                        insts.insert(pos, nop)
                        pos += 1


def _build_kernel(winplan, fix=True):
    """Per-core program.  winplan: per block, ([L2 chunks], [L3 chunks])."""
    nwin = sum(len(w2) + len(w3) for w2, w3 in winplan)
    nc = bass.Bass("TRN2", target_bir_lowering=False, num_devices=8,
                   num_swdge_queues=4)
    t01 = nc.dram_tensor("t01", [T01_ROWS, 2 * C], F16, kind="ExternalInput")
    # partition-major sequential loads: few DMAs, big per-partition descriptors
    t23d = nc.dram_tensor("t23", [128, T23_CHUNKS * C], F16,
                          kind="ExternalInput")
    w23d = nc.dram_tensor("w23", [128, nwin * 128], F16, kind="ExternalInput")
    idxd = nc.dram_tensor("idx", [128, IDXCOLS], I16, kind="ExternalInput")
    # wi = wts (NBLK*8 cols) ++ identity (128 cols)
    wid = nc.dram_tensor("wi", [128, NBLK * 8 + 128], F16,
                         kind="ExternalInput")
    outd = nc.dram_tensor("out", [3, 128, 8 * C], F16, kind="ExternalOutput")
    t01_h = t01[:].tensor

    with tile.TileContext(nc) as tc, ExitStack() as ctx:
        prep = ctx.enter_context(tc.tile_pool(name="prep", bufs=1))
        g0pool = ctx.enter_context(tc.tile_pool(name="g0", bufs=3))
        g1pool = ctx.enter_context(tc.tile_pool(name="g1", bufs=3))
        ltpool = ctx.enter_context(tc.tile_pool(name="lt", bufs=3))
        opool = ctx.enter_context(tc.tile_pool(name="o", bufs=1))
        ppool = ctx.enter_context(tc.tile_pool(name="ps", bufs=6, space="PSUM"))

        nc.gpsimd.load_library(library_config.attnmlp)

        idxt = prep.tile([128, IDXCOLS], I16, tag="idx")
        wit = prep.tile([128, NBLK * 8 + 128], F16, tag="wi")
        t23t = prep.tile([128, T23_CHUNKS, C], F16, tag="t23")
        w23t = prep.tile([128, nwin * 128], F16, tag="w23")
        warm = prep.tile([128, 1, C], F16, tag="warm")
        wtst = wit[:, 0 : NBLK * 8]
        identt = wit[:, NBLK * 8 : NBLK * 8 + 128]

        reg16 = nc.gpsimd.to_reg(16)
        reg512 = nc.gpsimd.to_reg(512)

        # warmup gather: no input dependency (idx = const-0 pool), absorbs
        # the cold-ucode cost while the tables stream in
        zidx = nc.const_aps.tensor(0.0, [128, 1], F32).bitcast(I16)[:, 0:1]
        nc.gpsimd.dma_gather(
            out_ap=warm[:],
            in_ap=bass.AP(t01_h, 0, [[C, 16], [1, C]]),
            idxs_ap=zidx,
            num_idxs=16,
            num_idxs_reg=reg16,
            elem_size=C,
            queue_num=0,
        )

        # sequential loads first: they need no gpsimd and fill the ramp
        nc.sync.dma_start(idxt[:], idxd[:])
        nc.sync.dma_start(wit[:], wid[:])
        nc.sync.dma_start(t23t[:], t23d[:])
        # w23 in first-use (block-major) order, split for parallelism
        half = (nwin + 1) // 2
        nc.sync.dma_start(w23t[:, 0 : half * 128], w23d[:, 0 : half * 128])
        nc.sync.dma_start(w23t[:, half * 128 : nwin * 128],
                          w23d[:, half * 128 : nwin * 128])

        prev_mm = None
        wslot = 0
        for sb in range(NSB):
            gts = []
            for l in range(2):
                W, H, base = L01[l]
                pool = g0pool if l == 0 else g1pool
                gt = pool.tile([128, 4, 4 * C], F16, tag=f"g{l}")
                # one descriptor = 4C elems spanning table rows r, r+1
                in_ap = bass.AP(t01_h, base * 2 * C,
                                [[2 * C, W * H], [1, 4 * C]])
                nc.gpsimd.dma_gather(
                    out_ap=gt[:],
                    in_ap=in_ap,
                    idxs_ap=idxt[:, (sb * 2 + l) * NGCOL
                                 : (sb * 2 + l + 1) * NGCOL],
                    num_idxs=512,
                    num_idxs_reg=reg512,
                    elem_size=4 * C,
                    elem_step=2 * C,
                    queue_num=(sb * 2 + l) % 4,
                )
                gts.append(gt)

            # diagonal lhsT for the 4 blocks of this superblock:
            # lt[q, i, j] = ident[q, j] * wts[q, sb*32 + i]
            lt = ltpool.tile([128, 32, 128], F16, tag="lt")
            nc.vector.tensor_tensor(
                lt[:],
                identt[:].unsqueeze(1).to_broadcast([128, 32, 128]),
                wtst[:, sb * 32 : (sb + 1) * 32]
                    .unsqueeze(2).to_broadcast([128, 32, 128]),
                AOP.mult,
            )

            for b4 in range(4):
                b = sb * 4 + b4
                w2, w3 = winplan[b]
                nmm = 8 + len(w2) + len(w3)
                ps = ppool.tile([128, C], F32, tag="ps")
                k = 0
                for l in range(2):
                    for k4 in range(4):
                        mm = nc.tensor.matmul(
                            ps[:],
                            lt[:, b4 * 8 + l * 4 + k4, :],
                            gts[l][:, b4, k4 * C : (k4 + 1) * C],
                            start=(k == 0),
                            stop=(k == nmm - 1),
                        )
                        if prev_mm is not None:
                            add_dep_helper(mm.ins, prev_mm.ins, sync=False)
                        prev_mm = mm
                        k += 1
                for chunks in (w2, w3):
                    for c in chunks:
                        mm = nc.tensor.matmul(
                            ps[:],
                            w23t[:, wslot * 128 : (wslot + 1) * 128],
                            t23t[:, c, :],
                            start=(k == 0),
                            stop=(k == nmm - 1),
                        )
                        add_dep_helper(mm.ins, prev_mm.ins, sync=False)
                        prev_mm = mm
                        k += 1
                        wslot += 1
                if b % 8 == 0:
                    ot = opool.tile([128, 8, C], F16, tag=f"ot{b // 8}")
                nc.scalar.activation(ot[:, b % 8, :], ps[:],
                                     mybir.ActivationFunctionType.Copy)
                if b % 8 == 7:
                    nc.sync.dma_start(outd[b // 8], ot[:])

    mybir.codegen_inst_isa_subclasses(nc)
    if fix:
        _fix_waits(nc)
    return nc


# ---------------------------------------------------------------------------
# Host-side prep

def _wrap128(flat):
    """Token-order idx list -> wrapped [16, n/16] replicated to [128, ...]."""
    w = flat.reshape(-1, 16).T.astype(np.int16)
    return np.tile(w, (8, 1))


def _points(center_b, boundary_b, roi0, nroi):
    """Flat sample-point coords (order h, w, roi) for one core."""
    bp = boundary_b[roi0 : roi0 + nroi]
    cp = center_b[roi0 : roi0 + nroi]
    sp = np.stack([bp[..., 0:2], cp, bp[..., 2:4]], axis=1)  # [nroi,3,Wp,2]
    gx = np.ascontiguousarray(sp[..., 0].transpose(1, 2, 0)).reshape(-1)
    gy = np.ascontiguousarray(sp[..., 1].transpose(1, 2, 0)).reshape(-1)
    return gx.astype(np.float32), gy.astype(np.float32)


def _lvl_geom(gx, gy, W, H):
    x = ((gx + np.float32(1.0)) * np.float32(0.5)) * np.float32(W - 1)
    y = ((gy + np.float32(1.0)) * np.float32(0.5)) * np.float32(H - 1)
    x0 = np.floor(x)
    y0 = np.floor(y)
    return x0.astype(np.int32), y0.astype(np.int32), x - x0, y - y0


class _CorePrep:
    """Per-core host data: sorted geometry for all 4 levels."""

    def __init__(self, center_b, boundary_b, roi0, nroi):
        gx, gy = _points(center_b, boundary_b, roi0, nroi)
        # sort by level-2 cell
        x2, y2, _, _ = _lvl_geom(gx, gy, 40, 40)
        self.sigma = np.argsort(y2 * 40 + x2, kind="stable")
        gx = gx[self.sigma]
        gy = gy[self.sigma]
        self.geo = {}
        for (W, H, _base) in L01 + L23:
            self.geo[W] = _lvl_geom(gx, gy, W, H)

    def corner_rows_weights(self, W):
        """[4, NPTS] corner table rows + weights (corner order 00,01,10,11)."""
        x0, y0, wx, wy = self.geo[W]
        r = y0 * W + x0
        rows = np.stack([r, r + W, r + 1, r + W + 1])
        wts = np.stack([(1 - wx) * (1 - wy), (1 - wx) * wy,
                        wx * (1 - wy), wx * wy])
        return rows, wts.astype(np.float32)

    def idx01_wts(self):
        idx = np.zeros((128, IDXCOLS), np.int16)
        wts = np.zeros((128, NBLK * 8), np.float16)
        for l, (W, H, _base) in enumerate(L01):
            x0, y0, wx, wy = self.geo[W]
            r = (y0 * W + x0).astype(np.int32)
            for sb in range(NSB):
                cs = (sb * 2 + l) * NGCOL
                idx[:, cs : cs + NGCOL] = _wrap128(r[sb * 512 : (sb + 1) * 512])
            _rows, w4 = self.corner_rows_weights(W)
            for b in range(NBLK):
                pts = slice(b * 128, (b + 1) * 128)
                for k in range(4):
                    wts[:, b * 8 + l * 4 + k] = w4[k, pts].astype(np.float16)
        return idx, wts

    def block_chunks(self, lvl):
        """Needed t23 chunks per block for level lvl (2 or 3)."""
        W, H, base = L23[lvl - 2]
        rows, wts = self.corner_rows_weights(W)
        out = []
        for b in range(NBLK):
            pts = slice(b * 128, (b + 1) * 128)
            rr = rows[:, pts] + base
            use = wts[:, pts] != 0.0
            out.append(set(np.unique(rr[use] // 128).tolist()))
        return out

    def w23(self, winplan):
        nwin = sum(len(w2) + len(w3) for w2, w3 in winplan)
        w23 = np.zeros((128, nwin * 128), np.float32)
        rw = {2: self.corner_rows_weights(40), 3: self.corner_rows_weights(20)}
        wslot = 0
        for b in range(NBLK):
            pts = slice(b * 128, (b + 1) * 128)
            for lvl, chunks in ((2, winplan[b][0]), (3, winplan[b][1])):
                rows, wts = rw[lvl]
                base = L23[lvl - 2][2]
                rr = rows[:, pts] + base           # [4, 128]
                ww = wts[:, pts]
                for c in chunks:
                    blkw = np.zeros((128, 128), np.float32)
                    rel = rr - c * 128
                    m = (rel >= 0) & (rel < 128) & (ww != 0.0)
                    jj = np.broadcast_to(np.arange(128)[None, :], (4, 128))
                    np.add.at(blkw, (rel[m], jj[m]), ww[m])
                    w23[:, wslot * 128 : (wslot + 1) * 128] = blkw
                    wslot += 1
        return w23.astype(np.float16)


def _host_t01(feats_b_list):
    """Row-pair table for levels 0/1: T01[y*W+x] = [row(y,x); row(y,x)+W]."""
    parts = []
    for f in feats_b_list:
        Cc, H, W = f.shape
        a = np.ascontiguousarray(f.reshape(Cc, -1).T)       # [H*W, C]
        bdown = np.concatenate([a[W:], np.zeros((W, Cc), a.dtype)], axis=0)
        parts.append(np.concatenate([a, bdown], axis=1))     # [H*W, 2C]
    t = np.concatenate(parts, axis=0)
    pad = T01_ROWS - t.shape[0]
    t = np.concatenate([t, np.zeros((pad, t.shape[1]), t.dtype)], axis=0)
    return np.ascontiguousarray(t.astype(np.float16))


def _host_t23(feats_b_list):
    """Plain channel-last tables for levels 2/3, chunk-padded,
    partition-major: out[p, k*C + c] = table[k*128 + p, c]."""
    t = np.zeros((T23_ROWS, C), np.float32)
    for f, (_W, _H, base) in zip(feats_b_list, L23):
        Cc, H, W = f.shape
        t[base : base + H * W] = f.reshape(Cc, -1).T
    t = t.reshape(T23_CHUNKS, 128, C).transpose(1, 0, 2)
    return np.ascontiguousarray(
        t.reshape(128, T23_CHUNKS * C).astype(np.float16))


_CACHE = {}


def kernel(feats0, feats1, feats2, feats3, center_points, boundary_points,
           _want_trace=False, _trace_dir=None):
    feats0 = np.asarray(feats0, dtype=np.float32)
    feats1 = np.asarray(feats1, dtype=np.float32)
    feats2 = np.asarray(feats2, dtype=np.float32)
    feats3 = np.asarray(feats3, dtype=np.float32)
    center_points = np.asarray(center_points, dtype=np.float32)
    boundary_points = np.asarray(boundary_points, dtype=np.float32)

    nroi = NROI_TOTAL // 4
    preps = []
    for core in range(8):
        b = core // 4
        roi0 = (core % 4) * nroi
        preps.append(_CorePrep(center_points[b], boundary_points[b],
                               roi0, nroi))

    # window plan: union over cores so one program fits all
    winplan = []
    per_core_chunks = [(p.block_chunks(2), p.block_chunks(3)) for p in preps]
    for b in range(NBLK):
        c2 = sorted(set().union(*[pc[0][b] for pc in per_core_chunks]))
        c3 = sorted(set().union(*[pc[1][b] for pc in per_core_chunks]))
        winplan.append((c2, c3))
    plan_key = tuple((tuple(w2), tuple(w3)) for w2, w3 in winplan)

    if _CACHE.get("key") != plan_key:
        _CACHE["nc"] = _build_kernel(winplan)
        _CACHE["key"] = plan_key
    nc = _CACHE["nc"]

    t01 = [_host_t01([feats0[b], feats1[b]]) for b in range(BS)]
    t23 = [_host_t23([feats2[b], feats3[b]]) for b in range(BS)]
    ident = np.eye(128, dtype=np.float16)

    in_maps = []
    for core in range(8):
        b = core // 4
        p = preps[core]
        idx, wts = p.idx01_wts()
        wi = np.concatenate([wts, ident], axis=1)
        in_maps.append({
            "t01": t01[b],
            "t23": t23[b],
            "w23": p.w23(winplan),
            "idx": idx,
            "wi": np.ascontiguousarray(wi),
        })

    kwargs = {}
    if _want_trace:
        kwargs = {"trace": True}
        if _trace_dir is not None:
            kwargs["tmpdir"] = _trace_dir
    res = run_bass_kernel_spmd(nc, in_maps, core_ids=list(range(8)), **kwargs)

    out = np.empty((BS, NROI_TOTAL, C, OUT_H, WP), np.float32)
    for core in range(8):
        b = core // 4
        roi0 = (core % 4) * nroi
        dev = res.results[core]["out"]          # [3, 128, 8*256] f16
        pts_sorted = (dev.astype(np.float32)
                      .reshape(3, 128, 8, C)
                      .transpose(0, 2, 1, 3)    # [3, 8, 128, C] = block, p
                      .reshape(NPTS, C))
        pts = np.empty_like(pts_sorted)
        pts[preps[core].sigma] = pts_sorted
        o = pts.reshape(OUT_H, WP, nroi, C)
        out[b, roi0 : roi0 + nroi] = o.transpose(2, 3, 0, 1)
    if _want_trace:
        return out, res
    return out
